# revision 33
# baseline (speedup 1.0000x reference)
"""Trainium2 Bass kernel for nn_AttentionLayer (B=4, S=2048, H=16, DH=64).

Sharding: 8 cores = 4 batches x 2 head-halves. Core c handles batch c//2,
heads (c%2)*8 .. (c%2)*8+8 (i.e. 512 of the 1024 QKV columns).

Per-core device program (SPMD, same program on all cores, different inputs):
  inputs (pre-laid-out on host):
    xT  [1024, 2048]  = x[b].T           (contraction dim on partitions)
    wq/wk/wv [1024, 512]                 (column slice for this core's heads)
    bq/bk/bv [512]
  output:
    out [512, 2048] = attention ctx for this core's 8 heads, transposed
                      (head*64+dh on rows, seq on cols); host transposes back.

Structure (single TileContext; all matmuls fp32r = full PE rate, ~1e-4 rel
error; Tile schedules by dependency + priority):
  - Priority bands: every attention-unit instruction outranks the QKV/V
    "filler" work, so the exp pipeline never starves while projections
    gap-fill the PE between attention matmuls.
  - V pass: V = x@Wv (PE) + bv (DVE add) -> v_aug [128, 8, 65] x16 seq tiles
    (col 64 = ones; the 65th PV output row accumulates the softmax
    denominator for free).
  - Per head-pair m: QT/KT = (x@W)^T on PE + bias via tensor_scalar_add on
    the PSUM->SBUF copy (weights streamed per pair, x re-streamed from HBM;
    Q chunks 2,3 deferred past qg0 to rebalance PE). Then attention units
    (m, qgroup of 1024 q, head):
      per k-block kb: scoresT [128,1024] = KT_chunk.T @ QT (PE, K=64);
                      E = exp(scoresT/8) (ACT, PSUM->SBUF, f32r out);
                      ctxT [65,512]x2 += v_aug.T @ E (PE, PSUM-accumulated)
      then per 512-q half: copy ctx to SBUF (frees PSUM), reciprocal of row
      64 (DVE), partition-broadcast (GPSIMD), multiply (DVE), DMA out.
  PSUM: scores 2x2 banks + ctx 2x1 + qkv 2 = 8. Modeled (TimelineSim)
  per-core time ~347 us; ACT (exp) busy ~267 us, PE busy ~305 us.
"""

import numpy as np

B, S, H, DH = 4, 2048, 16, 64
D = H * DH  # 1024
NCORES = 8
COLS = 512  # qkv columns per core (8 heads)
NPAIR = 4  # head pairs per core
NKB = S // 128  # 16 k-blocks
QG = 1024  # q-group width
NQG = S // QG  # 2
XC = 512  # x streaming chunk (seq cols)
NXC = S // XC  # 4
INV_SQRT_DH = 1.0 / 8.0

_CACHE = {}


def _build():
    import concourse.mybir as mybir
    import concourse.tile as tile
    from concourse import bacc

    f32 = mybir.dt.float32
    f32r = mybir.dt.float32r
    Exp = mybir.ActivationFunctionType.Exp

    nc = bacc.Bacc(
        "TRN2",
        target_bir_lowering=False,
        debug=False,
        enable_asserts=False,
        num_devices=NCORES,
    )

    xT_d = nc.dram_tensor("xT", [D, S], f32r, kind="ExternalInput").ap()
    wq_d = nc.dram_tensor("wq", [D, COLS], f32r, kind="ExternalInput").ap()
    wk_d = nc.dram_tensor("wk", [D, COLS], f32r, kind="ExternalInput").ap()
    wv_d = nc.dram_tensor("wv", [D, COLS], f32r, kind="ExternalInput").ap()
    bq_d = nc.dram_tensor("bq", [COLS], f32, kind="ExternalInput").ap()
    bk_d = nc.dram_tensor("bk", [COLS], f32, kind="ExternalInput").ap()
    bv_d = nc.dram_tensor("bv", [COLS], f32, kind="ExternalInput").ap()
    out_d = nc.dram_tensor("out", [COLS, S], f32, kind="ExternalOutput").ap()

    with tile.TileContext(nc) as tc:
        with (
            tc.tile_pool(name="consts", bufs=1) as consts,
            tc.tile_pool(name="vpool", bufs=1) as vpool,
            tc.tile_pool(name="wvpool", bufs=1) as wvpool,
            tc.tile_pool(name="wqk", bufs=2) as wqk,
            tc.tile_pool(name="xpool", bufs=2) as xpool,
            tc.tile_pool(name="qkt", bufs=2) as qkt,
            tc.tile_pool(name="epool", bufs=8) as epool,
            tc.tile_pool(name="opool", bufs=3) as opool,
            tc.tile_pool(name="psum", bufs=1, space="PSUM") as psum,
        ):
            # ---- constants, ACT table preload ----
            bq_t = consts.tile([128, NPAIR], f32)
            bk_t = consts.tile([128, NPAIR], f32)
            bv_s = consts.tile([1, COLS], f32)
            bvb = consts.tile([128, COLS], f32)
            nc.gpsimd.dma_start(out=bq_t, in_=bq_d.rearrange("(m p) -> p m", p=128))
            nc.gpsimd.dma_start(out=bk_t, in_=bk_d.rearrange("(m p) -> p m", p=128))
            nc.gpsimd.dma_start(out=bv_s, in_=bv_d[None, :])
            nc.gpsimd.partition_broadcast(bvb, bv_s)
            warm = consts.tile([1, 1], f32)
            nc.vector.memset(warm, 0.0)
            nc.scalar.activation(warm, warm, Exp)  # pull ACT table load early

            vt = [vpool.tile([128, 8, 65], f32r, name=f"vt{i}") for i in range(NKB)]
            for i in range(NKB):
                nc.vector.memset(vt[i][:, :, 64:65].bitcast(f32), 1.0)

            wv = wvpool.tile([128, 8, COLS], f32r, name="wv")

            def load_wv():
                nc.sync.dma_start(
                    out=wv, in_=wv_d.rearrange("(j p) c -> p j c", p=128)
                )

            def load_x_chunk(c, wpair=None):
                if wpair is not None:
                    load_w_dma(*wpair)
                xt = xpool.tile([128, 8, XC], f32r, name="xt", tag="xt")
                for j in range(8):
                    nc.sync.dma_start(
                        out=xt[:, j, :],
                        in_=xT_d[j * 128 : (j + 1) * 128, c * XC : (c + 1) * XC],
                    )
                return xt

            def v_pass(chunks):
                for c in chunks:
                    xt = load_x_chunk(c)
                    for i in range(XC // 128):
                        it = c * (XC // 128) + i
                        ps = psum.tile([128, 512], f32, tag="p1", bufs=2)
                        for j in range(8):
                            nc.tensor.matmul(
                                ps,
                                lhsT=xt[:, j, i * 128 : (i + 1) * 128],
                                rhs=wv[:, j, :],
                                start=(j == 0),
                                stop=(j == 7),
                            )
                        nc.vector.tensor_add(
                            vt[it][:, :, 0:64],
                            ps.rearrange("p (h d) -> p h d", h=8),
                            bvb.rearrange("p (h d) -> p h d", h=8),
                        )

            def load_w_dma(wqm, wkm, m):
                nc.sync.dma_start(
                    out=wqm,
                    in_=wq_d[:, m * 128 : (m + 1) * 128].rearrange(
                        "(j p) c -> p j c", p=128
                    ),
                )
                nc.sync.dma_start(
                    out=wkm,
                    in_=wk_d[:, m * 128 : (m + 1) * 128].rearrange(
                        "(j p) c -> p j c", p=128
                    ),
                )

            def load_w_pair(m, defer_dma=False):
                wqm = wqk.tile([128, 8, 128], f32r, name="wqm", tag="wqm")
                wkm = wqk.tile([128, 8, 128], f32r, name="wkm", tag="wkm")
                if not defer_dma:
                    load_w_dma(wqm, wkm, m)
                return wqm, wkm

            def qk_chunk(m, c, xt, wqm, wkm, qt, kt, projs=("q", "k")):
                pairs = {"q": (wqm, bq_t, qt), "k": (wkm, bk_t, kt)}
                for w, bias, dst in (pairs[p] for p in projs):
                    ps = psum.tile([128, 512], f32, tag="p1", bufs=2)
                    for j in range(8):
                        nc.tensor.matmul(
                            ps,
                            lhsT=w[:, j, :],
                            rhs=xt[:, j, :],
                            start=(j == 0),
                            stop=(j == 7),
                        )
                    nc.vector.tensor_scalar_add(
                        dst[:, c * XC : (c + 1) * XC], ps, bias[:, m : m + 1]
                    )

            def v_chunk(c, xt):
                for i in range(XC // 128):
                    it = c * (XC // 128) + i
                    ps = psum.tile([128, 512], f32, tag="p1", bufs=2)
                    for j in range(8):
                        nc.tensor.matmul(
                            ps,
                            lhsT=xt[:, j, i * 128 : (i + 1) * 128],
                            rhs=wv[:, j, :],
                            start=(j == 0),
                            stop=(j == 7),
                        )
                    nc.vector.tensor_add(
                        vt[it][:, :, 0:64],
                        ps.rearrange("p (h d) -> p h d", h=8),
                        bvb.rearrange("p (h d) -> p h d", h=8),
                    )

            def emit_attention_unit(m, qg, h, q0, p0, head, qt, kt):
                ctx = [
                    psum.tile([65, 512], f32, tag="ctx", bufs=2, name=f"ctx{qq}")
                    for qq in range(2)
                ]
                for kb in range(NKB):
                    sc = psum.tile([128, QG], f32, tag="sc", bufs=2)
                    for qq in range(2):
                        nc.tensor.matmul(
                            sc[:, qq * 512 : (qq + 1) * 512],
                            lhsT=kt[p0 : p0 + 64, kb * 128 : (kb + 1) * 128],
                            rhs=qt[
                                p0 : p0 + 64,
                                q0 + qq * 512 : q0 + (qq + 1) * 512,
                            ],
                            start=True,
                            stop=True,
                        )
                    ee = epool.tile([128, QG], f32r, tag="e")
                    nc.scalar.activation(ee, sc, Exp, scale=INV_SQRT_DH)
                    for qq in range(2):
                        nc.tensor.matmul(
                            ctx[qq],
                            lhsT=vt[kb][:, head, :],
                            rhs=ee[:, qq * 512 : (qq + 1) * 512],
                            start=(kb == 0),
                            stop=(kb == NKB - 1),
                        )
                for qq in range(2):
                    cs = opool.tile([65, 512], f32, tag="cs")
                    nc.vector.tensor_copy(cs, ctx[qq])
                    rr = opool.tile([1, 512], f32, tag="r")
                    nc.vector.reciprocal(rr, cs[64:65, :])
                    rb = opool.tile([64, 512], f32, tag="rb")
                    nc.gpsimd.partition_broadcast(rb, rr)
                    ob = opool.tile([64, 512], f32, tag="o")
                    nc.vector.tensor_mul(ob, cs[0:64, :], rb)
                    nc.sync.dma_start(
                        out=out_d[
                            head * 64 : (head + 1) * 64,
                            q0 + qq * 512 : q0 + (qq + 1) * 512,
                        ],
                        in_=ob,
                    )

            # ---- banded priorities: attention preferred, QKV/V fill gaps ----
            from contextlib import contextmanager

            base = tc.cur_priority + 50
            att_cur = [base]
            fill_cur = [base + 6000]

            @contextmanager
            def band(cursor):
                off = tc.cur_priority - cursor[0]
                with tc.high_priority(offset=off):
                    yield
                    cursor[0] = tc.cur_priority

            # ---- per pair: QKV (filler band) then attention (att band) ----
            for m in range(NPAIR):
                with band(fill_cur):
                    wqm, wkm = load_w_pair(m, defer_dma=(m == 0))
                    qt = qkt.tile([128, S], f32r, name=f"qt{m}", tag="qt")
                    kt = qkt.tile([128, S], f32r, name=f"kt{m}", tag="kt")
                    xts = {}
                    if m == 0:
                        xts[0] = load_x_chunk(0, wpair=(wqm, wkm, m))
                        xts[1] = load_x_chunk(1)
                        qk_chunk(m, 0, xts[0], wqm, wkm, qt, kt)
                        qk_chunk(m, 1, xts[1], wqm, wkm, qt, kt)
                        load_wv()
                        v_chunk(0, xts[0])
                        v_chunk(1, xts[1])
                        for c in (2, 3):
                            xts[c] = load_x_chunk(c)
                            qk_chunk(m, c, xts[c], wqm, wkm, qt, kt, projs=("k",))
                            v_chunk(c, xts[c])
                    else:
                        for c in range(NXC):
                            xts[c] = load_x_chunk(c)
                            qk_chunk(
                                m, c, xts[c], wqm, wkm, qt, kt,
                                projs=("q", "k") if c < 2 else ("k",),
                            )

                # ---- attention units (Q c2/c3 deferred after qg0) ----
                for qg in range(NQG):
                    q0 = qg * QG
                    for h in range(2):
                        head = 2 * m + h
                        p0 = h * 64
                        with band(att_cur):
                            emit_attention_unit(m, qg, h, q0, p0, head, qt, kt)
                    if qg == 0:
                        with band(fill_cur):
                            for c in (2, 3):
                                qk_chunk(m, c, xts[c], wqm, wkm, qt, kt, projs=("q",))


    nc.compile()
    return nc


def _get_nc():
    if "nc" not in _CACHE:
        _CACHE["nc"] = _build()
    return _CACHE["nc"]


def _in_maps(x, Wq, bq, Wk, bk, Wv, bv):
    maps = []
    for c in range(NCORES):
        b, hh = c // 2, c % 2
        cs = slice(hh * COLS, (hh + 1) * COLS)
        maps.append(
            {
                "xT": np.ascontiguousarray(np.asarray(x)[b].T),
                "wq": np.ascontiguousarray(np.asarray(Wq)[:, cs]),
                "wk": np.ascontiguousarray(np.asarray(Wk)[:, cs]),
                "wv": np.ascontiguousarray(np.asarray(Wv)[:, cs]),
                "bq": np.ascontiguousarray(np.asarray(bq)[cs]),
                "bk": np.ascontiguousarray(np.asarray(bk)[cs]),
                "bv": np.ascontiguousarray(np.asarray(bv)[cs]),
            }
        )
    return maps


def _run(inputs, trace=False):
    from concourse import bass_utils

    nc = _get_nc()
    res = bass_utils.run_bass_kernel_spmd(
        nc,
        _in_maps(**inputs),
        core_ids=list(range(NCORES)),
        trace=trace,
    )
    out = np.empty((B, S, D), np.float32)
    for c in range(NCORES):
        b, hh = c // 2, c % 2
        out[b, :, hh * COLS : (hh + 1) * COLS] = res.results[c]["out"].T
    return out, res


def kernel(**inputs):
    out, _ = _run(inputs, trace=False)
    return out


if __name__ == "__main__":
    _get_nc()
    print("build ok")
